# revision 2
# baseline (speedup 1.0000x reference)
"""Trainium2 Bass kernel for nn_GAT_47906065220065 — dst-stationary redesign.

SSGConv (K=1, alpha=0.5) -> GATv2(12 heads x 12) -> GATv2(1 head x 64),
N=100000 nodes (padded 102400), E=1e6 edges + self loops.

Nodes are sorted by in-degree into 800 blocks of 128; blocks are dealt
round-robin to the 8 cores (core = g % 8); tableid(node) = core*12800 +
(g//8)*128 + slot.  Per block, edge slot (p, j) holds the j-th in-edge of
dst p, so gathered source rows land already dst-aligned: no one-hot S
matrices and no transposes in the inner loop.  The per-block trip count K
comes from core 0's block of the same local index (sorted order makes it the
max across cores); runs of equal K share one hardware For_i loop (strata).

Per edge column j: INDIRECT1D gather of the source row from the bf16 table,
z = gxl + xrd (DVE), lrelu = max(z, 0.2z) (Act scale-copy + DVE max),
logits = reduce(lrelu*att) (DVE), ex = exp(logit + mb_j) (Act; per-column
bias -SHIFT real / -50 pad), then numerator/denominator via buffered writes
+ one block-end reduce (layer 1) or a diagonal-matmul PSUM accumulate
(layer 2).  The self-loop column is a direct DMA of the block's own
contiguous rows of the local table shard.  Softmax uses the constant shift
exp(l - 4) (logits bounded ~[-5.3, 4.1] for this data).

Projections build only the core's own 12800 table rows (feature-major lhsT,
no transposes) and AllGather the bf16 table; bias1 is folded into b2l/b2r.
Phase 1's gather is precomputed on the host into a sequential, dinv-
prescaled bf16 edge stream (no indirect DMA at all in phase 1).
"""

import os
import sys

sys.path.insert(0, '/opt/trn_rl_repo')

import numpy as np

import bass_rust
import concourse.bacc as bacc
import concourse.bass as bass
import concourse.mybir as mybir
import concourse.tile as tile
from concourse.bass_utils import run_bass_kernel_spmd

N = 100000
NPAD = 102400
NCORES = 8
PERCORE = NPAD // NCORES          # 12800
NBLK = PERCORE // 128             # 100
D_IN = 64
F1 = 144
H1, C1 = 12, 12
F2 = 64
ALPHA = 0.5
SHIFT = 4.0
MASK = -50.0
PHASES = int(os.environ.get("KERNEL2_PHASES", "5"))
FP = mybir.dt.float32
BF = mybir.dt.bfloat16
I32 = mybir.dt.int32

AF = mybir.ActivationFunctionType
ALU = mybir.AluOpType


# ------------------------------------------------- walrus compatibility
def _drain_and_barrier_split(self, tick_clock, wait_clock):
    from concourse.vector_clock import ScopedClock

    carrier = self.nc.sync.nop(nofuse=True)
    wait_clock.add_sem_waits(
        carrier.ins, ScopedClock({None: tick_clock.global_clock})
    )
    si0 = carrier.ins.sync_info
    waits = list(si0.on_wait or []) if si0 is not None else []
    if len(waits) > 1:
        carrier.ins.sync_info = bass_rust.SyncInfo(
            on_wait=waits[:1], on_update=list(si0.on_update or [])
        )
        for w in waits[1:]:
            extra = self.nc.sync.nop(nofuse=True)
            extra.ins.sync_info = bass_rust.SyncInfo(on_wait=[w], on_update=[])
    self.nc.sync.drain()

    self.nc.all_engine_barrier()
    assert self.sems is not None
    popped = self.nc._tile_sem_poison_stack.pop()
    assert popped is self._sem_poison
    self.nc.clear_and_free_semaphores(list(self.sems.allocated().values()))
    self.nc.all_engine_barrier()


tile.TileContext._drain_and_barrier = _drain_and_barrier_split

_WSPLIT_N = [0]


def _split_sync_waits(nc):
    def make_nop(engine, wait):
        _WSPLIT_N[0] += 1
        return mybir.InstNoOp(
            name=f"WSPLIT-{_WSPLIT_N[0]}", opcode="NoOp", engine=engine,
            debug=None, ins=[], outs=[], descendants=None,
            sync_info=bass_rust.SyncInfo(on_wait=[wait], on_update=[]),
            bass_sim_breakpoint=False, bass_priority=0,
            bass_wait_until_ts=None, bass_scheduled_tick=None,
            bass_scheduled_proc=None, bass_scheduled_scope=None,
            bass_addl_debug=None, text_hint=None, bass_nofuse=True,
        )

    for f in nc.m.functions:
        for bb in f.blocks:
            if not any(
                inst.sync_info and inst.sync_info.on_wait
                and len(inst.sync_info.on_wait) > 1
                for inst in bb.instructions
            ):
                continue
            new_insts = []
            for inst in bb.instructions:
                si = inst.sync_info
                waits = list(si.on_wait) if si and si.on_wait else []
                if len(waits) > 1:
                    for w in waits[:-1]:
                        new_insts.append(make_nop(inst.engine, w))
                    inst.sync_info = bass_rust.SyncInfo(
                        on_wait=[waits[-1]], on_update=list(si.on_update or [])
                    )
                new_insts.append(inst)
            bb.instructions = new_insts


def _bf16(x):
    import ml_dtypes
    return np.ascontiguousarray(np.asarray(x, dtype=np.float32)).astype(
        ml_dtypes.bfloat16)


# ------------------------------------------------------------ host prep
def _host_prep(features, edge_index, params):
    x = np.ascontiguousarray(np.asarray(features), dtype=np.float32)
    ei = np.asarray(edge_index)
    src = ei[0].astype(np.int64)
    dst = ei[1].astype(np.int64)

    indeg = np.bincount(dst, minlength=NPAD)        # excl self loops
    deg = indeg.copy()
    deg[:N] += 1                                    # + self loop (real only)
    dinv = np.zeros(NPAD, np.float32)
    nz = deg > 0
    dinv[nz] = 1.0 / np.sqrt(deg[nz].astype(np.float32))

    order = np.argsort(-indeg, kind='stable')       # NPAD ranks desc
    ranks = np.arange(NPAD)
    g_of = ranks // 128
    tid = np.empty(NPAD, np.int64)
    tid[order] = (g_of % NCORES) * PERCORE + (g_of // NCORES) * 128 \
        + (ranks % 128)

    blk_first = indeg[order[::128]]                 # [800] in g order
    Khat = blk_first.reshape(NBLK, NCORES)[:, 0].astype(np.int64)
    assert (Khat[:, None] >= blk_first.reshape(NBLK, NCORES)).all()
    JTOT = int(Khat.sum())
    ofs = np.zeros(NBLK, np.int64)
    ofs[1:] = np.cumsum(Khat)[:-1]
    ofsm = np.zeros(NBLK, np.int64)
    ofsm[1:] = np.cumsum(Khat + 1)[:-1]
    MTOT = JTOT + NBLK

    strata = []
    lo = 0
    for lb in range(1, NBLK + 1):
        if lb == NBLK or Khat[lb] != Khat[lo]:
            strata.append((lo, lb, int(Khat[lo])))
            lo = lb
    sig = tuple(strata)

    # per-dst edge lists in tableid space (vectorized)
    tsrc = tid[src]
    tdst = tid[dst]
    eorder = np.argsort(tdst, kind='stable')
    tsrc_s = tsrc[eorder]
    tdst_s = tdst[eorder]
    counts = np.bincount(tdst_s, minlength=NPAD)
    starts = np.zeros(NPAD, np.int64)
    starts[1:] = np.cumsum(counts)[:-1]
    within = np.arange(len(tsrc_s)) - starts[tdst_s]

    d_core = tdst_s // PERCORE
    d_loc = tdst_s % PERCORE
    d_lb = d_loc // 128
    d_p = d_loc % 128

    srcs_arr = np.zeros((NCORES, 128, JTOT), np.int32)
    srcs_arr[d_core, d_p, ofs[d_lb] + within] = tsrc_s.astype(np.int32)
    mb_arr = np.full((NCORES, 128, MTOT), MASK, np.float32)
    mb_arr[d_core, d_p, ofsm[d_lb] + within] = -SHIFT
    # self col (index K within each block's mb range) for every dst
    all_t = np.arange(NPAD)
    a_core = all_t // PERCORE
    a_loc = all_t % PERCORE
    a_lb = a_loc // 128
    a_p = a_loc % 128
    mb_arr[a_core, a_p, ofsm[a_lb] + Khat[a_lb]] = -SHIFT

    # p1 stream column sources: self at ofsm[lb], edges at ofsm[lb]+1+within
    xs_cols = np.zeros((NCORES, 128, MTOT), np.int64)
    xs_msk = np.zeros((NCORES, 128, MTOT), np.float32)
    xs_cols[a_core, a_p, ofsm[a_lb]] = all_t
    xs_msk[a_core, a_p, ofsm[a_lb]] = 1.0
    xs_cols[d_core, d_p, ofsm[d_lb] + 1 + within] = tsrc_s
    xs_msk[d_core, d_p, ofsm[d_lb] + 1 + within] = 1.0

    # x and dinv in tableid order; stream values prescaled by dinv[src]
    xp = np.zeros((NPAD, D_IN), np.float32)
    xp[tid[:N]] = x
    dinv_t = np.zeros(NPAD, np.float32)
    dinv_t[tid] = dinv
    xs_tab = xp * dinv_t[:, None]

    g = lambda k: np.ascontiguousarray(np.asarray(params[k]), dtype=np.float32)
    W_ssg, b_ssg = g('W_ssg'), g('b_ssg')
    W1l, b1l, W1r, b1r = g('W1l'), g('b1l'), g('W1r'), g('b1r')
    att1, bias1 = g('att1'), g('bias1')
    W2l, b2l, W2r, b2r = g('W2l'), g('b2l'), g('W2r'), g('b2r')
    att2, bias2 = g('att2'), g('bias2')
    b2l_adj = b2l + bias1 @ W2l
    b2r_adj = b2r + bias1 @ W2r

    w2la = np.zeros((128, 65), np.float32)
    w2la[:, :64] = W2l[:128]
    w2lb = np.zeros((16, 65), np.float32)
    w2lb[:, :64] = W2l[128:]
    b2lrow = np.zeros((1, 65), np.float32)
    b2lrow[0, :64] = b2l_adj
    b2lrow[0, 64] = 1.0

    consts = dict(
        iota=np.tile(np.arange(128, dtype=np.float32), (128, 1)),
        iotac=np.arange(128, dtype=np.float32).reshape(128, 1),
        identf=np.eye(128, dtype=np.float32),
        ident=_bf16(np.eye(128, dtype=np.float32)),
        ones1=_bf16(np.ones((1, 128), np.float32)),
        wssg=_bf16(W_ssg),
        bssg=np.ascontiguousarray(b_ssg.reshape(64, 1)),
        w1l=_bf16(W1l), b1lr=_bf16(b1l.reshape(1, F1)),
        w1r=_bf16(W1r), b1rr=_bf16(b1r.reshape(1, F1)),
        att1r=_bf16(np.tile(att1.reshape(1, F1), (128, 1))),
        w2la=_bf16(w2la), w2lb=_bf16(w2lb), b2lrow=_bf16(b2lrow),
        w2ra=_bf16(W2r[:128]), w2rb=_bf16(W2r[128:]),
        b2rrow=_bf16(b2r_adj.reshape(1, F2)),
        att2r=_bf16(np.tile(att2.reshape(1, F2), (128, 1))),
        bias2r=np.ascontiguousarray(
            np.tile(bias2.reshape(1, F2), (128, 1)).astype(np.float32)),
    )

    in_maps = []
    for c in range(NCORES):
        m = dict(consts)
        m['srcs'] = np.ascontiguousarray(srcs_arr[c])
        m['mb'] = np.ascontiguousarray(mb_arr[c])
        vals = xs_tab[xs_cols[c].reshape(-1)].reshape(128, MTOT, D_IN)
        vals *= xs_msk[c][:, :, None]
        m['xs'] = _bf16(vals.reshape(128, MTOT * D_IN))
        own = slice(c * PERCORE, (c + 1) * PERCORE)
        m['xa'] = np.ascontiguousarray(ALPHA * xp[own])
        m['s1'] = np.ascontiguousarray(
            ((1.0 - ALPHA) * dinv_t[own]).reshape(PERCORE, 1))
        in_maps.append(m)

    return in_maps, tid, sig, JTOT, MTOT


# --------------------------------------------------------- kernel build
def _build(sig, JTOT, MTOT):
    nc = bacc.Bacc()

    srcs = nc.declare_dram_parameter("srcs", [128, JTOT], I32, isOutput=False)
    mbp = nc.declare_dram_parameter("mb", [128, MTOT], FP, isOutput=False)
    xs = nc.declare_dram_parameter("xs", [128, MTOT * D_IN], BF,
                                   isOutput=False)
    xa = nc.declare_dram_parameter("xa", [PERCORE, D_IN], FP, isOutput=False)
    s1 = nc.declare_dram_parameter("s1", [PERCORE, 1], FP, isOutput=False)

    cshape = dict(
        iota=([128, 128], FP), iotac=([128, 1], FP),
        identf=([128, 128], FP), ident=([128, 128], BF),
        ones1=([1, 128], BF),
        wssg=([64, 64], BF), bssg=([64, 1], FP),
        w1l=([64, F1], BF), b1lr=([1, F1], BF),
        w1r=([64, F1], BF), b1rr=([1, F1], BF),
        att1r=([128, F1], BF),
        w2la=([128, 65], BF), w2lb=([16, 65], BF), b2lrow=([1, 65], BF),
        w2ra=([128, F2], BF), w2rb=([16, F2], BF), b2rrow=([1, F2], BF),
        att2r=([128, F2], BF), bias2r=([128, F2], FP),
    )
    cparams = {k: nc.declare_dram_parameter(k, shp, dt, isOutput=False)
               for k, (shp, dt) in cshape.items()}

    out = nc.declare_dram_parameter("out", [PERCORE, F2], FP, isOutput=True)

    x1T_d = nc.dram_tensor("x1T_d", [D_IN, PERCORE], BF)
    y1T_d = nc.dram_tensor("y1T_d", [F1, PERCORE], BF)
    xl1_own = nc.dram_tensor("xl1_own", [PERCORE, F1], BF)
    xl1_all = nc.dram_tensor("xl1_all", [NPAD, F1], BF, addr_space="Shared")
    xl2_own = nc.dram_tensor("xl2_own", [PERCORE, 65], BF)
    xl2_all = nc.dram_tensor("xl2_all", [NPAD, 65], BF, addr_space="Shared")

    ds = bass.ds
    rg = [list(range(NCORES))]
    KMAX = max(k for lo, hi, k in sig)

    with tile.TileContext(nc) as tc:
        cpool = tc.alloc_tile_pool(name="consts", bufs=1)
        ct = {}
        for k, (shp, dt) in cshape.items():
            ct[k] = cpool.tile(shp, dt, tag=f"c_{k}", name=f"c_{k}")
            nc.sync.dma_start(out=ct[k][:], in_=cparams[k][:])

        # ---------------- phase 1: SSG conv -> x1T_d ------------------
        with (tc.tile_pool(name="p1s", bufs=4) as pool,
              tc.tile_pool(name="p1a", bufs=2, space="PSUM") as ppa,
              tc.tile_pool(name="p1t", bufs=2, space="PSUM") as ppt):
            for b in range(NBLK):
                K1 = int(_KHAT[b]) + 1
                xo = int(_OFSM[b])
                st = pool.tile([128, (KMAX + 1) * D_IN], BF, tag="xs")
                nc.sync.dma_start(out=st[:, :K1 * 64],
                                    in_=xs[:, ds(xo * 64, K1 * 64)])
                acc = ppa.tile([128, D_IN], FP, tag="acc")
                for j in range(K1):
                    nc.tensor.matmul(
                        acc[:], lhsT=ct['ident'][:],
                        rhs=st[:, j * 64:(j + 1) * 64],
                        start=(j == 0), stop=(j == K1 - 1))
                s1t = pool.tile([128, 1], FP, tag="s1t")
                nc.scalar.dma_start(out=s1t[:], in_=s1[ds(b * 128, 128), :])
                xat = pool.tile([128, D_IN], FP, tag="xat")
                nc.scalar.dma_start(out=xat[:], in_=xa[ds(b * 128, 128), :])
                t1 = pool.tile([128, D_IN], FP, tag="t1")
                nc.scalar.activation(t1[:], acc[:], AF.Identity,
                                     scale=s1t[:, :1])
                x1h = pool.tile([128, D_IN], FP, tag="x1h")
                nc.vector.tensor_tensor(out=x1h[:], in0=t1[:], in1=xat[:],
                                        op=ALU.add)
                hTp = ppt.tile([64, 128], FP, tag="hTp")
                nc.tensor.transpose(out=hTp[:], in_=x1h[:],
                                    identity=ct['identf'][:])
                hTs = pool.tile([64, 128], BF, tag="hTs")
                nc.scalar.activation(hTs[:], hTp[:], AF.Copy)
                x1p = ppt.tile([64, 128], FP, tag="x1p")
                nc.tensor.matmul(x1p[:], lhsT=ct['wssg'][:], rhs=hTs[:],
                                 start=True, stop=True)
                x1s = pool.tile([64, 128], BF, tag="x1s")
                nc.scalar.activation(x1s[:], x1p[:], AF.Identity,
                                     bias=ct['bssg'][:, :1])
                nc.scalar.dma_start(out=x1T_d[:, ds(b * 128, 128)],
                                    in_=x1s[:])
                ps2 = ppt.tile([128, F1], FP, tag="ps2")
                nc.tensor.matmul(ps2[:], lhsT=x1s[:], rhs=ct['w1l'][:],
                                 start=True, stop=False)
                nc.tensor.matmul(ps2[:], lhsT=ct['ones1'][:],
                                 rhs=ct['b1lr'][:], start=False, stop=True)
                xls = pool.tile([128, F1], BF, tag="xls")
                nc.scalar.activation(xls[:], ps2[:], AF.Copy)
                nc.scalar.dma_start(out=xl1_own[ds(b * 128, 128), :],
                                    in_=xls[:])

        if PHASES >= 3:
            nc.gpsimd.collective_compute(
                "AllGather", ALU.bypass, replica_groups=rg,
                ins=[xl1_own[:]], outs=[xl1_all[:]])

        # ---------------- phase 3: GATv2 layer 1 -> y1T_d -------------
        if PHASES >= 3:
         with (tc.tile_pool(name="p3s", bufs=8) as pool,
              tc.tile_pool(name="p3t", bufs=2, space="PSUM") as ppt):
            for b in range(NBLK):
                K = int(_KHAT[b])
                cso = int(_OFS[b])
                cmo = int(_OFSM[b])
                if True:
                    if K > 0:
                        stg_i = pool.tile([128, KMAX], I32, tag="stg_i")
                        nc.sync.dma_start(out=stg_i[:, :K],
                                            in_=srcs[:, ds(cso, K)])
                    stg_m = pool.tile([128, KMAX + 1], FP, tag="stg_m")
                    nc.sync.dma_start(out=stg_m[:, :K + 1],
                                        in_=mbp[:, ds(cmo, K + 1)])
                    x1b = pool.tile([64, 128], BF, tag="x1b3")
                    nc.sync.dma_start(out=x1b[:],
                                        in_=x1T_d[:, ds(b * 128, 128)])
                    psR = ppt.tile([128, F1], FP, tag="psR")
                    nc.tensor.matmul(psR[:], lhsT=x1b[:], rhs=ct['w1r'][:],
                                     start=True, stop=False)
                    nc.tensor.matmul(psR[:], lhsT=ct['ones1'][:],
                                     rhs=ct['b1rr'][:], start=False, stop=True)
                    xrds = pool.tile([128, F1], BF, tag="xrds")
                    nc.scalar.activation(xrds[:], psR[:], AF.Copy)
                    xld = pool.tile([128, F1], BF, tag="xld")
                    nc.scalar.dma_start(out=xld[:],
                                        in_=xl1_own[ds(b * 128, 128), :])
                    exb = pool.tile([128, (KMAX + 1) * H1], BF, tag="exb")
                    prodb = pool.tile([128, (KMAX + 1) * F1], BF,
                                      tag="prodb")
                    for j in range(K + 1):
                        if j < K:
                            gxl = pool.tile([128, F1], BF, tag="gxl")
                            nc.gpsimd.indirect_dma_start(
                                out=gxl[:], out_offset=None, in_=xl1_all[:],
                                in_offset=bass.IndirectOffsetOnAxis(
                                    ap=stg_i[:, j:j + 1], axis=0))
                        else:
                            gxl = xld
                        z = pool.tile([128, F1], BF, tag="z")
                        nc.vector.tensor_tensor(out=z[:], in0=gxl[:],
                                                in1=xrds[:], op=ALU.add)
                        lr = pool.tile([128, F1], BF, tag="lr")
                        nc.scalar.activation(lr[:], z[:], AF.Prelu, alpha=0.2)
                        wm = pool.tile([128, F1], BF, tag="wm")
                        nc.vector.tensor_tensor(out=wm[:], in0=lr[:],
                                                in1=ct['att1r'][:],
                                                op=ALU.mult)
                        lg = pool.tile([128, H1], FP, tag="lg")
                        nc.vector.tensor_reduce(
                            out=lg[:],
                            in_=wm[:].rearrange("p (h c) -> p h c", c=C1),
                            axis=mybir.AxisListType.X, op=ALU.add)
                        nc.scalar.activation(
                            exb[:, j * H1:(j + 1) * H1], lg[:], AF.Exp,
                            bias=stg_m[:, j:j + 1])
                        nc.vector.tensor_tensor(
                            out=prodb[:, j * F1:(j + 1) * F1].rearrange(
                                "p (h c) -> p h c", c=C1),
                            in0=gxl[:].rearrange("p (h c) -> p h c", c=C1),
                            in1=exb[:, j * H1:(j + 1) * H1].to_broadcast(
                                [128, H1, C1]),
                            op=ALU.mult)
                    den = pool.tile([128, H1], FP, tag="den")
                    nc.vector.tensor_reduce(
                        out=den[:],
                        in_=exb[:, :(K + 1) * H1].rearrange(
                            "p (j h) -> p h j", h=H1),
                        axis=mybir.AxisListType.X, op=ALU.add)
                    rec = pool.tile([128, H1], FP, tag="rec")
                    nc.vector.reciprocal(rec[:], den[:])
                    num = pool.tile([128, F1], FP, tag="num")
                    nc.vector.tensor_reduce(
                        out=num[:],
                        in_=prodb[:, :(K + 1) * F1].rearrange(
                            "p (j c) -> p c j", c=F1),
                        axis=mybir.AxisListType.X, op=ALU.add)
                    y1 = pool.tile([128, F1], BF, tag="y1")
                    nc.vector.tensor_tensor(
                        out=y1[:].rearrange("p (h c) -> p h c", c=C1),
                        in0=num[:].rearrange("p (h c) -> p h c", c=C1),
                        in1=rec[:].to_broadcast([128, H1, C1]), op=ALU.mult)
                    tY1 = ppt.tile([128, 128], BF, tag="tY1")
                    nc.tensor.transpose(out=tY1[:], in_=y1[:, :128],
                                        identity=ct['ident'][:])
                    tY2 = ppt.tile([16, 128], BF, tag="tY2")
                    nc.tensor.transpose(out=tY2[:], in_=y1[:, 128:F1],
                                        identity=ct['ident'][:])
                    sY1 = pool.tile([128, 128], BF, tag="sY1")
                    nc.scalar.activation(sY1[:], tY1[:], AF.Copy)
                    sY2 = pool.tile([16, 128], BF, tag="sY2")
                    nc.scalar.activation(sY2[:], tY2[:], AF.Copy)
                    nc.scalar.dma_start(out=y1T_d[:128, ds(b * 128, 128)],
                                        in_=sY1[:])
                    nc.scalar.dma_start(out=y1T_d[128:F1, ds(b * 128, 128)],
                                        in_=sY2[:])
                    ps4 = ppt.tile([128, 65], FP, tag="ps4")
                    nc.tensor.matmul(ps4[:], lhsT=sY1[:], rhs=ct['w2la'][:],
                                     start=True, stop=False)
                    nc.tensor.matmul(ps4[:], lhsT=sY2[:], rhs=ct['w2lb'][:],
                                     start=False, stop=False)
                    nc.tensor.matmul(ps4[:], lhsT=ct['ones1'][:],
                                     rhs=ct['b2lrow'][:], start=False,
                                     stop=True)
                    xls4 = pool.tile([128, 65], BF, tag="xls4")
                    nc.scalar.activation(xls4[:], ps4[:], AF.Copy)
                    nc.scalar.dma_start(out=xl2_own[ds(b * 128, 128), :],
                                        in_=xls4[:])

        if PHASES >= 5:
            nc.gpsimd.collective_compute(
                "AllGather", ALU.bypass, replica_groups=rg,
                ins=[xl2_own[:]], outs=[xl2_all[:]])

        # ---------------- phase 5: GATv2 layer 2 -> out ---------------
        if PHASES >= 5:
         with (tc.tile_pool(name="p5s", bufs=8) as pool,
              tc.tile_pool(name="p5a", bufs=2, space="PSUM") as ppa,
              tc.tile_pool(name="p5t", bufs=2, space="PSUM") as ppt):
            for b in range(NBLK):
                K = int(_KHAT[b])
                cso = int(_OFS[b])
                cmo = int(_OFSM[b])
                if True:
                    if K > 0:
                        stg_i = pool.tile([128, KMAX], I32, tag="stg_i5")
                        nc.sync.dma_start(out=stg_i[:, :K],
                                            in_=srcs[:, ds(cso, K)])
                    stg_m = pool.tile([128, KMAX + 1], FP, tag="stg_m5")
                    nc.sync.dma_start(out=stg_m[:, :K + 1],
                                        in_=mbp[:, ds(cmo, K + 1)])
                    y1b1 = pool.tile([128, 128], BF, tag="y1b15")
                    nc.sync.dma_start(out=y1b1[:],
                                        in_=y1T_d[:128, ds(b * 128, 128)])
                    y1b2 = pool.tile([16, 128], BF, tag="y1b25")
                    nc.sync.dma_start(out=y1b2[:],
                                        in_=y1T_d[128:F1, ds(b * 128, 128)])
                    psR = ppt.tile([128, F2], FP, tag="psR5")
                    nc.tensor.matmul(psR[:], lhsT=y1b1[:], rhs=ct['w2ra'][:],
                                     start=True, stop=False)
                    nc.tensor.matmul(psR[:], lhsT=y1b2[:], rhs=ct['w2rb'][:],
                                     start=False, stop=False)
                    nc.tensor.matmul(psR[:], lhsT=ct['ones1'][:],
                                     rhs=ct['b2rrow'][:], start=False,
                                     stop=True)
                    xrd2 = pool.tile([128, F2], BF, tag="xrd2")
                    nc.scalar.activation(xrd2[:], psR[:], AF.Copy)
                    xld2 = pool.tile([128, 65], BF, tag="xld2")
                    nc.scalar.dma_start(out=xld2[:],
                                        in_=xl2_own[ds(b * 128, 128), :])
                    prodb = pool.tile([128, (KMAX + 1) * 65], BF,
                                      tag="prodb5")
                    for j in range(K + 1):
                        if j < K:
                            gx2 = pool.tile([128, 65], BF, tag="gx2")
                            nc.gpsimd.indirect_dma_start(
                                out=gx2[:], out_offset=None, in_=xl2_all[:],
                                in_offset=bass.IndirectOffsetOnAxis(
                                    ap=stg_i[:, j:j + 1], axis=0))
                        else:
                            gx2 = xld2
                        z = pool.tile([128, F2], BF, tag="z5")
                        nc.vector.tensor_tensor(out=z[:], in0=gx2[:, :64],
                                                in1=xrd2[:], op=ALU.add)
                        lr = pool.tile([128, F2], BF, tag="lr5")
                        nc.scalar.activation(lr[:], z[:], AF.Prelu, alpha=0.2)
                        wm = pool.tile([128, F2], BF, tag="wm5")
                        nc.vector.tensor_tensor(out=wm[:], in0=lr[:],
                                                in1=ct['att2r'][:],
                                                op=ALU.mult)
                        lg = pool.tile([128, 1], FP, tag="lg5")
                        nc.vector.tensor_reduce(
                            out=lg[:],
                            in_=wm[:].rearrange("p (h c) -> p h c", c=F2),
                            axis=mybir.AxisListType.X, op=ALU.add)
                        ext = pool.tile([128, 1], FP, tag="ext")
                        nc.scalar.activation(ext[:], lg[:], AF.Exp,
                                             bias=stg_m[:, j:j + 1])
                        nc.vector.tensor_scalar(
                            prodb[:, j * 65:(j + 1) * 65], gx2[:],
                            ext[:, :1], None, op0=ALU.mult)
                    num2 = pool.tile([128, 65], FP, tag="num2")
                    nc.vector.tensor_reduce(
                        out=num2[:],
                        in_=prodb[:, :(K + 1) * 65].rearrange(
                            "p (j c) -> p c j", c=65),
                        axis=mybir.AxisListType.X, op=ALU.add)
                    rec = pool.tile([128, 1], FP, tag="rec5")
                    nc.vector.reciprocal(rec[:], num2[:, 64:65])
                    o1 = pool.tile([128, F2], FP, tag="o1")
                    nc.vector.tensor_scalar(o1[:], num2[:, :64],
                                            rec[:, :1], None, op0=ALU.mult)
                    o2 = pool.tile([128, F2], FP, tag="o2")
                    nc.vector.tensor_tensor(out=o2[:], in0=o1[:],
                                            in1=ct['bias2r'][:], op=ALU.add)
                    nc.scalar.dma_start(out=out[ds(b * 128, 128), :],
                                        in_=o2[:])

        if PHASES < 5:
            with tc.tile_pool(name="zf", bufs=1) as zp:
                zt = zp.tile([128, F2], FP, tag="zt", name="zt")
                nc.vector.memset(zt[:], 0.0)
                for b in range(NBLK):
                    nc.sync.dma_start(out=out[ds(b * 128, 128), :], in_=zt[:])
        cpool.release()

    nc.compile()
    _split_sync_waits(nc)
    return nc


_OFS = None
_OFSM = None
_KHAT = None
_NC_CACHE = {}


def _prepare(inputs):
    global _OFS, _OFSM, _KHAT
    in_maps, tid, sig, JTOT, MTOT = _host_prep(
        inputs["features"], inputs["edge_index"], inputs)
    khat = np.array([k for lo, hi, k in sig for _ in range(hi - lo)],
                    np.int64)
    _OFS = np.zeros(NBLK, np.int64)
    _OFS[1:] = np.cumsum(khat)[:-1]
    _OFSM = np.zeros(NBLK, np.int64)
    _OFSM[1:] = np.cumsum(khat + 1)[:-1]
    _KHAT = khat
    key = (sig, PHASES)
    if key not in _NC_CACHE:
        _NC_CACHE[key] = _build(sig, JTOT, MTOT)
    return _NC_CACHE[key], in_maps, tid


def kernel(**inputs):
    nc, in_maps, tid = _prepare(inputs)
    res = run_bass_kernel_spmd(nc, in_maps, list(range(NCORES)))
    full = np.concatenate([res.results[c]["out"] for c in range(NCORES)],
                          axis=0)
    return np.ascontiguousarray(full[tid[:N]]).astype(np.float32)


# revision 3
# speedup vs baseline: 1.0608x; 1.0608x over previous
"""Trainium2 Bass kernel for nn_GAT_47906065220065 — dst-stationary redesign.

SSGConv (K=1, alpha=0.5) -> GATv2(12 heads x 12) -> GATv2(1 head x 64),
N=100000 nodes (padded 102400), E=1e6 edges + self loops.

Nodes are sorted by in-degree into 800 blocks of 128; blocks are dealt
round-robin to the 8 cores (core = g % 8); tableid(node) = core*12800 +
(g//8)*128 + slot.  Per block, edge slot (p, j) holds the j-th in-edge of
dst p, so gathered source rows land already dst-aligned: no one-hot S
matrices and no transposes in the inner loop.  The per-block trip count K
comes from core 0's block of the same local index (sorted order makes it the
max across cores); runs of equal K share one hardware For_i loop (strata).

Per edge column j: INDIRECT1D gather of the source row from the bf16 table,
z = gxl + xrd (DVE), lrelu = max(z, 0.2z) (Act scale-copy + DVE max),
logits = reduce(lrelu*att) (DVE), ex = exp(logit + mb_j) (Act; per-column
bias -SHIFT real / -50 pad), then numerator/denominator via buffered writes
+ one block-end reduce (layer 1) or a diagonal-matmul PSUM accumulate
(layer 2).  The self-loop column is a direct DMA of the block's own
contiguous rows of the local table shard.  Softmax uses the constant shift
exp(l - 4) (logits bounded ~[-5.3, 4.1] for this data).

Projections build only the core's own 12800 table rows (feature-major lhsT,
no transposes) and AllGather the bf16 table; bias1 is folded into b2l/b2r.
Phase 1's gather is precomputed on the host into a sequential, dinv-
prescaled bf16 edge stream (no indirect DMA at all in phase 1).
"""

import os
import sys

sys.path.insert(0, '/opt/trn_rl_repo')

import numpy as np

import bass_rust
import concourse.bacc as bacc
import concourse.bass as bass
import concourse.mybir as mybir
import concourse.tile as tile
from concourse.bass_utils import run_bass_kernel_spmd

N = 100000
NPAD = 102400
NCORES = 8
PERCORE = NPAD // NCORES          # 12800
NBLK = PERCORE // 128             # 100
D_IN = 64
F1 = 144
H1, C1 = 12, 12
F2 = 64
ALPHA = 0.5
SHIFT = 4.0
MASK = -50.0
PHASES = int(os.environ.get("KERNEL2_PHASES", "5"))
FP = mybir.dt.float32
BF = mybir.dt.bfloat16
I32 = mybir.dt.int32

AF = mybir.ActivationFunctionType
ALU = mybir.AluOpType


# ------------------------------------------------- walrus compatibility
def _drain_and_barrier_split(self, tick_clock, wait_clock):
    from concourse.vector_clock import ScopedClock

    carrier = self.nc.sync.nop(nofuse=True)
    wait_clock.add_sem_waits(
        carrier.ins, ScopedClock({None: tick_clock.global_clock})
    )
    si0 = carrier.ins.sync_info
    waits = list(si0.on_wait or []) if si0 is not None else []
    if len(waits) > 1:
        carrier.ins.sync_info = bass_rust.SyncInfo(
            on_wait=waits[:1], on_update=list(si0.on_update or [])
        )
        for w in waits[1:]:
            extra = self.nc.sync.nop(nofuse=True)
            extra.ins.sync_info = bass_rust.SyncInfo(on_wait=[w], on_update=[])
    self.nc.sync.drain()

    self.nc.all_engine_barrier()
    assert self.sems is not None
    popped = self.nc._tile_sem_poison_stack.pop()
    assert popped is self._sem_poison
    self.nc.clear_and_free_semaphores(list(self.sems.allocated().values()))
    self.nc.all_engine_barrier()


tile.TileContext._drain_and_barrier = _drain_and_barrier_split

_WSPLIT_N = [0]


def _split_sync_waits(nc):
    def make_nop(engine, wait):
        _WSPLIT_N[0] += 1
        return mybir.InstNoOp(
            name=f"WSPLIT-{_WSPLIT_N[0]}", opcode="NoOp", engine=engine,
            debug=None, ins=[], outs=[], descendants=None,
            sync_info=bass_rust.SyncInfo(on_wait=[wait], on_update=[]),
            bass_sim_breakpoint=False, bass_priority=0,
            bass_wait_until_ts=None, bass_scheduled_tick=None,
            bass_scheduled_proc=None, bass_scheduled_scope=None,
            bass_addl_debug=None, text_hint=None, bass_nofuse=True,
        )

    for f in nc.m.functions:
        for bb in f.blocks:
            if not any(
                inst.sync_info and inst.sync_info.on_wait
                and len(inst.sync_info.on_wait) > 1
                for inst in bb.instructions
            ):
                continue
            new_insts = []
            for inst in bb.instructions:
                si = inst.sync_info
                waits = list(si.on_wait) if si and si.on_wait else []
                if len(waits) > 1:
                    for w in waits[:-1]:
                        new_insts.append(make_nop(inst.engine, w))
                    inst.sync_info = bass_rust.SyncInfo(
                        on_wait=[waits[-1]], on_update=list(si.on_update or [])
                    )
                new_insts.append(inst)
            bb.instructions = new_insts


def _bf16(x):
    import ml_dtypes
    return np.ascontiguousarray(np.asarray(x, dtype=np.float32)).astype(
        ml_dtypes.bfloat16)


# ------------------------------------------------------------ host prep
def _host_prep(features, edge_index, params):
    x = np.ascontiguousarray(np.asarray(features), dtype=np.float32)
    ei = np.asarray(edge_index)
    src = ei[0].astype(np.int64)
    dst = ei[1].astype(np.int64)

    indeg = np.bincount(dst, minlength=NPAD)        # excl self loops
    deg = indeg.copy()
    deg[:N] += 1                                    # + self loop (real only)
    dinv = np.zeros(NPAD, np.float32)
    nz = deg > 0
    dinv[nz] = 1.0 / np.sqrt(deg[nz].astype(np.float32))

    order = np.argsort(-indeg, kind='stable')       # NPAD ranks desc
    ranks = np.arange(NPAD)
    g_of = ranks // 128
    tid = np.empty(NPAD, np.int64)
    tid[order] = (g_of % NCORES) * PERCORE + (g_of // NCORES) * 128 \
        + (ranks % 128)

    blk_first = indeg[order[::128]]                 # [800] in g order
    Khat = blk_first.reshape(NBLK, NCORES)[:, 0].astype(np.int64)
    assert (Khat[:, None] >= blk_first.reshape(NBLK, NCORES)).all()
    JTOT = int(Khat.sum())
    ofs = np.zeros(NBLK, np.int64)
    ofs[1:] = np.cumsum(Khat)[:-1]
    ofsm = np.zeros(NBLK, np.int64)
    ofsm[1:] = np.cumsum(Khat + 1)[:-1]
    MTOT = JTOT + NBLK

    strata = []
    lo = 0
    for lb in range(1, NBLK + 1):
        if lb == NBLK or Khat[lb] != Khat[lo]:
            strata.append((lo, lb, int(Khat[lo])))
            lo = lb
    sig = tuple(strata)

    # per-dst edge lists in tableid space (vectorized)
    tsrc = tid[src]
    tdst = tid[dst]
    eorder = np.argsort(tdst, kind='stable')
    tsrc_s = tsrc[eorder]
    tdst_s = tdst[eorder]
    counts = np.bincount(tdst_s, minlength=NPAD)
    starts = np.zeros(NPAD, np.int64)
    starts[1:] = np.cumsum(counts)[:-1]
    within = np.arange(len(tsrc_s)) - starts[tdst_s]

    d_core = tdst_s // PERCORE
    d_loc = tdst_s % PERCORE
    d_lb = d_loc // 128
    d_p = d_loc % 128

    srcs_arr = np.zeros((NCORES, 128, JTOT), np.int32)
    srcs_arr[d_core, d_p, ofs[d_lb] + within] = tsrc_s.astype(np.int32)
    mb_arr = np.full((NCORES, 128, MTOT), MASK, np.float32)
    mb_arr[d_core, d_p, ofsm[d_lb] + within] = -SHIFT
    # self col (index K within each block's mb range) for every dst
    all_t = np.arange(NPAD)
    a_core = all_t // PERCORE
    a_loc = all_t % PERCORE
    a_lb = a_loc // 128
    a_p = a_loc % 128
    mb_arr[a_core, a_p, ofsm[a_lb] + Khat[a_lb]] = -SHIFT

    # p1 stream column sources: self at ofsm[lb], edges at ofsm[lb]+1+within
    xs_cols = np.zeros((NCORES, 128, MTOT), np.int64)
    xs_msk = np.zeros((NCORES, 128, MTOT), np.float32)
    xs_cols[a_core, a_p, ofsm[a_lb]] = all_t
    xs_msk[a_core, a_p, ofsm[a_lb]] = 1.0
    xs_cols[d_core, d_p, ofsm[d_lb] + 1 + within] = tsrc_s
    xs_msk[d_core, d_p, ofsm[d_lb] + 1 + within] = 1.0

    # x and dinv in tableid order; stream values prescaled by dinv[src]
    xp = np.zeros((NPAD, D_IN), np.float32)
    xp[tid[:N]] = x
    dinv_t = np.zeros(NPAD, np.float32)
    dinv_t[tid] = dinv
    xs_tab = xp * dinv_t[:, None]

    g = lambda k: np.ascontiguousarray(np.asarray(params[k]), dtype=np.float32)
    W_ssg, b_ssg = g('W_ssg'), g('b_ssg')
    W1l, b1l, W1r, b1r = g('W1l'), g('b1l'), g('W1r'), g('b1r')
    att1, bias1 = g('att1'), g('bias1')
    W2l, b2l, W2r, b2r = g('W2l'), g('b2l'), g('W2r'), g('b2r')
    att2, bias2 = g('att2'), g('bias2')
    b2l_adj = b2l + bias1 @ W2l
    b2r_adj = b2r + bias1 @ W2r

    w2la = np.zeros((128, 65), np.float32)
    w2la[:, :64] = W2l[:128]
    w2lb = np.zeros((16, 65), np.float32)
    w2lb[:, :64] = W2l[128:]
    b2lrow = np.zeros((1, 65), np.float32)
    b2lrow[0, :64] = b2l_adj
    b2lrow[0, 64] = 1.0

    consts = dict(
        iota=np.tile(np.arange(128, dtype=np.float32), (128, 1)),
        iotac=np.arange(128, dtype=np.float32).reshape(128, 1),
        identf=np.eye(128, dtype=np.float32),
        ident=_bf16(np.eye(128, dtype=np.float32)),
        ones1=_bf16(np.ones((1, 128), np.float32)),
        wssg=_bf16(W_ssg),
        bssg=np.ascontiguousarray(b_ssg.reshape(64, 1)),
        w1l=_bf16(W1l), b1lr=_bf16(b1l.reshape(1, F1)),
        w1r=_bf16(W1r), b1rr=_bf16(b1r.reshape(1, F1)),
        att1r=_bf16(np.tile(att1.reshape(1, F1), (128, 1))),
        w2la=_bf16(w2la), w2lb=_bf16(w2lb), b2lrow=_bf16(b2lrow),
        w2ra=_bf16(W2r[:128]), w2rb=_bf16(W2r[128:]),
        b2rrow=_bf16(b2r_adj.reshape(1, F2)),
        att2r=_bf16(np.tile(att2.reshape(1, F2), (128, 1))),
        bias2r=np.ascontiguousarray(
            np.tile(bias2.reshape(1, F2), (128, 1)).astype(np.float32)),
    )

    in_maps = []
    for c in range(NCORES):
        m = dict(consts)
        m['srcs'] = np.ascontiguousarray(srcs_arr[c])
        m['mb'] = np.ascontiguousarray(mb_arr[c])
        vals = xs_tab[xs_cols[c].reshape(-1)].reshape(128, MTOT, D_IN)
        vals *= xs_msk[c][:, :, None]
        m['xs'] = _bf16(vals.reshape(128, MTOT * D_IN))
        own = slice(c * PERCORE, (c + 1) * PERCORE)
        m['xa'] = np.ascontiguousarray(ALPHA * xp[own])
        m['s1'] = np.ascontiguousarray(
            ((1.0 - ALPHA) * dinv_t[own]).reshape(PERCORE, 1))
        in_maps.append(m)

    return in_maps, tid, sig, JTOT, MTOT


# --------------------------------------------------------- kernel build
def _build(sig, JTOT, MTOT):
    nc = bacc.Bacc()

    srcs = nc.declare_dram_parameter("srcs", [128, JTOT], I32, isOutput=False)
    mbp = nc.declare_dram_parameter("mb", [128, MTOT], FP, isOutput=False)
    xs = nc.declare_dram_parameter("xs", [128, MTOT * D_IN], BF,
                                   isOutput=False)
    xa = nc.declare_dram_parameter("xa", [PERCORE, D_IN], FP, isOutput=False)
    s1 = nc.declare_dram_parameter("s1", [PERCORE, 1], FP, isOutput=False)

    cshape = dict(
        iota=([128, 128], FP), iotac=([128, 1], FP),
        identf=([128, 128], FP), ident=([128, 128], BF),
        ones1=([1, 128], BF),
        wssg=([64, 64], BF), bssg=([64, 1], FP),
        w1l=([64, F1], BF), b1lr=([1, F1], BF),
        w1r=([64, F1], BF), b1rr=([1, F1], BF),
        att1r=([128, F1], BF),
        w2la=([128, 65], BF), w2lb=([16, 65], BF), b2lrow=([1, 65], BF),
        w2ra=([128, F2], BF), w2rb=([16, F2], BF), b2rrow=([1, F2], BF),
        att2r=([128, F2], BF), bias2r=([128, F2], FP),
    )
    cparams = {k: nc.declare_dram_parameter(k, shp, dt, isOutput=False)
               for k, (shp, dt) in cshape.items()}

    out = nc.declare_dram_parameter("out", [PERCORE, F2], FP, isOutput=True)

    x1T_d = nc.dram_tensor("x1T_d", [D_IN, PERCORE], BF)
    y1T_d = nc.dram_tensor("y1T_d", [F1, PERCORE], BF)
    xl1_own = nc.dram_tensor("xl1_own", [PERCORE, F1], BF)
    xl1_all = nc.dram_tensor("xl1_all", [NPAD, F1], BF, addr_space="Shared")
    xl2_own = nc.dram_tensor("xl2_own", [PERCORE, 65], BF)
    xl2_all = nc.dram_tensor("xl2_all", [NPAD, 65], BF, addr_space="Shared")

    ds = bass.ds
    rg = [list(range(NCORES))]
    KMAX = max(k for lo, hi, k in sig)

    with tile.TileContext(nc) as tc:
        cpool = tc.alloc_tile_pool(name="consts", bufs=1)
        ct = {}
        for k, (shp, dt) in cshape.items():
            ct[k] = cpool.tile(shp, dt, tag=f"c_{k}", name=f"c_{k}")
            nc.sync.dma_start(out=ct[k][:], in_=cparams[k][:])

        # ---------------- phase 1: SSG conv -> x1T_d ------------------
        with (tc.tile_pool(name="p1s", bufs=4) as pool,
              tc.tile_pool(name="p1a", bufs=2, space="PSUM") as ppa,
              tc.tile_pool(name="p1t", bufs=2, space="PSUM") as ppt):
            for b in range(NBLK):
                K1 = int(_KHAT[b]) + 1
                xo = int(_OFSM[b])
                st = pool.tile([128, (KMAX + 1) * D_IN], BF, tag="xs")
                nc.sync.dma_start(out=st[:, :K1 * 64],
                                    in_=xs[:, ds(xo * 64, K1 * 64)])
                acc = ppa.tile([128, D_IN], FP, tag="acc")
                for j in range(K1):
                    nc.tensor.matmul(
                        acc[:], lhsT=ct['ident'][:],
                        rhs=st[:, j * 64:(j + 1) * 64],
                        start=(j == 0), stop=(j == K1 - 1))
                s1t = pool.tile([128, 1], FP, tag="s1t")
                nc.scalar.dma_start(out=s1t[:], in_=s1[ds(b * 128, 128), :])
                xat = pool.tile([128, D_IN], FP, tag="xat")
                nc.scalar.dma_start(out=xat[:], in_=xa[ds(b * 128, 128), :])
                t1 = pool.tile([128, D_IN], FP, tag="t1")
                nc.scalar.activation(t1[:], acc[:], AF.Identity,
                                     scale=s1t[:, :1])
                x1h = pool.tile([128, D_IN], FP, tag="x1h")
                nc.vector.tensor_tensor(out=x1h[:], in0=t1[:], in1=xat[:],
                                        op=ALU.add)
                hTp = ppt.tile([64, 128], FP, tag="hTp")
                nc.tensor.transpose(out=hTp[:], in_=x1h[:],
                                    identity=ct['identf'][:])
                hTs = pool.tile([64, 128], BF, tag="hTs")
                nc.scalar.activation(hTs[:], hTp[:], AF.Copy)
                x1p = ppt.tile([64, 128], FP, tag="x1p")
                nc.tensor.matmul(x1p[:], lhsT=ct['wssg'][:], rhs=hTs[:],
                                 start=True, stop=True)
                x1s = pool.tile([64, 128], BF, tag="x1s")
                nc.scalar.activation(x1s[:], x1p[:], AF.Identity,
                                     bias=ct['bssg'][:, :1])
                nc.scalar.dma_start(out=x1T_d[:, ds(b * 128, 128)],
                                    in_=x1s[:])
                ps2 = ppt.tile([128, F1], FP, tag="ps2")
                nc.tensor.matmul(ps2[:], lhsT=x1s[:], rhs=ct['w1l'][:],
                                 start=True, stop=False)
                nc.tensor.matmul(ps2[:], lhsT=ct['ones1'][:],
                                 rhs=ct['b1lr'][:], start=False, stop=True)
                xls = pool.tile([128, F1], BF, tag="xls")
                nc.scalar.activation(xls[:], ps2[:], AF.Copy)
                nc.scalar.dma_start(out=xl1_own[ds(b * 128, 128), :],
                                    in_=xls[:])

        if PHASES >= 3:
            nc.gpsimd.collective_compute(
                "AllGather", ALU.bypass, replica_groups=rg,
                ins=[xl1_own[:]], outs=[xl1_all[:]])

        # ---------------- phase 3: GATv2 layer 1 -> y1T_d -------------
        if PHASES >= 3:
         with (tc.tile_pool(name="p3s", bufs=8) as pool,
              tc.tile_pool(name="p3g", bufs=16) as gpool,
              tc.tile_pool(name="p3t", bufs=2, space="PSUM") as ppt):
            for b in range(NBLK):
                K = int(_KHAT[b])
                cso = int(_OFS[b])
                cmo = int(_OFSM[b])
                if True:
                    if K > 0:
                        stg_i = pool.tile([128, KMAX], I32, tag="stg_i")
                        nc.sync.dma_start(out=stg_i[:, :K],
                                            in_=srcs[:, ds(cso, K)])
                    stg_m = pool.tile([128, KMAX + 1], FP, tag="stg_m")
                    nc.sync.dma_start(out=stg_m[:, :K + 1],
                                        in_=mbp[:, ds(cmo, K + 1)])
                    x1b = pool.tile([64, 128], BF, tag="x1b3")
                    nc.sync.dma_start(out=x1b[:],
                                        in_=x1T_d[:, ds(b * 128, 128)])
                    psR = ppt.tile([128, F1], FP, tag="psR")
                    nc.tensor.matmul(psR[:], lhsT=x1b[:], rhs=ct['w1r'][:],
                                     start=True, stop=False)
                    nc.tensor.matmul(psR[:], lhsT=ct['ones1'][:],
                                     rhs=ct['b1rr'][:], start=False, stop=True)
                    xrds = pool.tile([128, F1], BF, tag="xrds")
                    nc.scalar.activation(xrds[:], psR[:], AF.Copy)
                    xld = pool.tile([128, F1], BF, tag="xld")
                    nc.scalar.dma_start(out=xld[:],
                                        in_=xl1_own[ds(b * 128, 128), :])
                    exb = pool.tile([128, (KMAX + 1) * H1], BF, tag="exb")
                    prodb = pool.tile([128, (KMAX + 1) * F1], BF,
                                      tag="prodb")
                    for j in range(K + 1):
                        if j < K:
                            gxl = gpool.tile([128, F1], BF, tag="gxl")
                            nc.gpsimd.indirect_dma_start(
                                out=gxl[:], out_offset=None, in_=xl1_all[:],
                                in_offset=bass.IndirectOffsetOnAxis(
                                    ap=stg_i[:, j:j + 1], axis=0))
                        else:
                            gxl = xld
                        z = pool.tile([128, F1], BF, tag="z")
                        nc.vector.tensor_tensor(out=z[:], in0=gxl[:],
                                                in1=xrds[:], op=ALU.add)
                        lr = pool.tile([128, F1], BF, tag="lr")
                        nc.scalar.activation(lr[:], z[:], AF.Prelu, alpha=0.2)
                        wm = pool.tile([128, F1], BF, tag="wm")
                        nc.vector.tensor_tensor(out=wm[:], in0=lr[:],
                                                in1=ct['att1r'][:],
                                                op=ALU.mult)
                        lg = pool.tile([128, H1], FP, tag="lg")
                        nc.vector.tensor_reduce(
                            out=lg[:],
                            in_=wm[:].rearrange("p (h c) -> p h c", c=C1),
                            axis=mybir.AxisListType.X, op=ALU.add)
                        nc.scalar.activation(
                            exb[:, j * H1:(j + 1) * H1], lg[:], AF.Exp,
                            bias=stg_m[:, j:j + 1])
                        nc.vector.tensor_tensor(
                            out=prodb[:, j * F1:(j + 1) * F1].rearrange(
                                "p (h c) -> p h c", c=C1),
                            in0=gxl[:].rearrange("p (h c) -> p h c", c=C1),
                            in1=exb[:, j * H1:(j + 1) * H1].to_broadcast(
                                [128, H1, C1]),
                            op=ALU.mult)
                    den = pool.tile([128, H1], FP, tag="den")
                    nc.vector.tensor_reduce(
                        out=den[:],
                        in_=exb[:, :(K + 1) * H1].rearrange(
                            "p (j h) -> p h j", h=H1),
                        axis=mybir.AxisListType.X, op=ALU.add)
                    rec = pool.tile([128, H1], FP, tag="rec")
                    nc.vector.reciprocal(rec[:], den[:])
                    num = pool.tile([128, F1], FP, tag="num")
                    nc.vector.tensor_reduce(
                        out=num[:],
                        in_=prodb[:, :(K + 1) * F1].rearrange(
                            "p (j c) -> p c j", c=F1),
                        axis=mybir.AxisListType.X, op=ALU.add)
                    y1 = pool.tile([128, F1], BF, tag="y1")
                    nc.vector.tensor_tensor(
                        out=y1[:].rearrange("p (h c) -> p h c", c=C1),
                        in0=num[:].rearrange("p (h c) -> p h c", c=C1),
                        in1=rec[:].to_broadcast([128, H1, C1]), op=ALU.mult)
                    tY1 = ppt.tile([128, 128], BF, tag="tY1")
                    nc.tensor.transpose(out=tY1[:], in_=y1[:, :128],
                                        identity=ct['ident'][:])
                    tY2 = ppt.tile([16, 128], BF, tag="tY2")
                    nc.tensor.transpose(out=tY2[:], in_=y1[:, 128:F1],
                                        identity=ct['ident'][:])
                    sY1 = pool.tile([128, 128], BF, tag="sY1")
                    nc.scalar.activation(sY1[:], tY1[:], AF.Copy)
                    sY2 = pool.tile([16, 128], BF, tag="sY2")
                    nc.scalar.activation(sY2[:], tY2[:], AF.Copy)
                    nc.scalar.dma_start(out=y1T_d[:128, ds(b * 128, 128)],
                                        in_=sY1[:])
                    nc.scalar.dma_start(out=y1T_d[128:F1, ds(b * 128, 128)],
                                        in_=sY2[:])
                    ps4 = ppt.tile([128, 65], FP, tag="ps4")
                    nc.tensor.matmul(ps4[:], lhsT=sY1[:], rhs=ct['w2la'][:],
                                     start=True, stop=False)
                    nc.tensor.matmul(ps4[:], lhsT=sY2[:], rhs=ct['w2lb'][:],
                                     start=False, stop=False)
                    nc.tensor.matmul(ps4[:], lhsT=ct['ones1'][:],
                                     rhs=ct['b2lrow'][:], start=False,
                                     stop=True)
                    xls4 = pool.tile([128, 65], BF, tag="xls4")
                    nc.scalar.activation(xls4[:], ps4[:], AF.Copy)
                    nc.scalar.dma_start(out=xl2_own[ds(b * 128, 128), :],
                                        in_=xls4[:])

        if PHASES >= 5:
            nc.gpsimd.collective_compute(
                "AllGather", ALU.bypass, replica_groups=rg,
                ins=[xl2_own[:]], outs=[xl2_all[:]])

        # ---------------- phase 5: GATv2 layer 2 -> out ---------------
        if PHASES >= 5:
         with (tc.tile_pool(name="p5s", bufs=8) as pool,
              tc.tile_pool(name="p5g", bufs=16) as gpool,
              tc.tile_pool(name="p5a", bufs=2, space="PSUM") as ppa,
              tc.tile_pool(name="p5t", bufs=2, space="PSUM") as ppt):
            for b in range(NBLK):
                K = int(_KHAT[b])
                cso = int(_OFS[b])
                cmo = int(_OFSM[b])
                if True:
                    if K > 0:
                        stg_i = pool.tile([128, KMAX], I32, tag="stg_i5")
                        nc.sync.dma_start(out=stg_i[:, :K],
                                            in_=srcs[:, ds(cso, K)])
                    stg_m = pool.tile([128, KMAX + 1], FP, tag="stg_m5")
                    nc.sync.dma_start(out=stg_m[:, :K + 1],
                                        in_=mbp[:, ds(cmo, K + 1)])
                    y1b1 = pool.tile([128, 128], BF, tag="y1b15")
                    nc.sync.dma_start(out=y1b1[:],
                                        in_=y1T_d[:128, ds(b * 128, 128)])
                    y1b2 = pool.tile([16, 128], BF, tag="y1b25")
                    nc.sync.dma_start(out=y1b2[:],
                                        in_=y1T_d[128:F1, ds(b * 128, 128)])
                    psR = ppt.tile([128, F2], FP, tag="psR5")
                    nc.tensor.matmul(psR[:], lhsT=y1b1[:], rhs=ct['w2ra'][:],
                                     start=True, stop=False)
                    nc.tensor.matmul(psR[:], lhsT=y1b2[:], rhs=ct['w2rb'][:],
                                     start=False, stop=False)
                    nc.tensor.matmul(psR[:], lhsT=ct['ones1'][:],
                                     rhs=ct['b2rrow'][:], start=False,
                                     stop=True)
                    xrd2 = pool.tile([128, F2], BF, tag="xrd2")
                    nc.scalar.activation(xrd2[:], psR[:], AF.Copy)
                    xld2 = pool.tile([128, 65], BF, tag="xld2")
                    nc.scalar.dma_start(out=xld2[:],
                                        in_=xl2_own[ds(b * 128, 128), :])
                    prodb = pool.tile([128, (KMAX + 1) * 65], BF,
                                      tag="prodb5")
                    for j in range(K + 1):
                        if j < K:
                            gx2 = gpool.tile([128, 65], BF, tag="gx2")
                            nc.gpsimd.indirect_dma_start(
                                out=gx2[:], out_offset=None, in_=xl2_all[:],
                                in_offset=bass.IndirectOffsetOnAxis(
                                    ap=stg_i[:, j:j + 1], axis=0))
                        else:
                            gx2 = xld2
                        z = pool.tile([128, F2], BF, tag="z5")
                        nc.vector.tensor_tensor(out=z[:], in0=gx2[:, :64],
                                                in1=xrd2[:], op=ALU.add)
                        lr = pool.tile([128, F2], BF, tag="lr5")
                        nc.scalar.activation(lr[:], z[:], AF.Prelu, alpha=0.2)
                        wm = pool.tile([128, F2], BF, tag="wm5")
                        nc.vector.tensor_tensor(out=wm[:], in0=lr[:],
                                                in1=ct['att2r'][:],
                                                op=ALU.mult)
                        lg = pool.tile([128, 1], FP, tag="lg5")
                        nc.vector.tensor_reduce(
                            out=lg[:],
                            in_=wm[:].rearrange("p (h c) -> p h c", c=F2),
                            axis=mybir.AxisListType.X, op=ALU.add)
                        ext = pool.tile([128, 1], FP, tag="ext")
                        nc.scalar.activation(ext[:], lg[:], AF.Exp,
                                             bias=stg_m[:, j:j + 1])
                        nc.vector.tensor_scalar(
                            prodb[:, j * 65:(j + 1) * 65], gx2[:],
                            ext[:, :1], None, op0=ALU.mult)
                    num2 = pool.tile([128, 65], FP, tag="num2")
                    nc.vector.tensor_reduce(
                        out=num2[:],
                        in_=prodb[:, :(K + 1) * 65].rearrange(
                            "p (j c) -> p c j", c=65),
                        axis=mybir.AxisListType.X, op=ALU.add)
                    rec = pool.tile([128, 1], FP, tag="rec5")
                    nc.vector.reciprocal(rec[:], num2[:, 64:65])
                    o1 = pool.tile([128, F2], FP, tag="o1")
                    nc.vector.tensor_scalar(o1[:], num2[:, :64],
                                            rec[:, :1], None, op0=ALU.mult)
                    o2 = pool.tile([128, F2], FP, tag="o2")
                    nc.vector.tensor_tensor(out=o2[:], in0=o1[:],
                                            in1=ct['bias2r'][:], op=ALU.add)
                    nc.scalar.dma_start(out=out[ds(b * 128, 128), :],
                                        in_=o2[:])

        if PHASES < 5:
            with tc.tile_pool(name="zf", bufs=1) as zp:
                zt = zp.tile([128, F2], FP, tag="zt", name="zt")
                nc.vector.memset(zt[:], 0.0)
                for b in range(NBLK):
                    nc.sync.dma_start(out=out[ds(b * 128, 128), :], in_=zt[:])
        cpool.release()

    nc.compile()
    _split_sync_waits(nc)
    return nc


_OFS = None
_OFSM = None
_KHAT = None
_NC_CACHE = {}


def _prepare(inputs):
    global _OFS, _OFSM, _KHAT
    in_maps, tid, sig, JTOT, MTOT = _host_prep(
        inputs["features"], inputs["edge_index"], inputs)
    khat = np.array([k for lo, hi, k in sig for _ in range(hi - lo)],
                    np.int64)
    _OFS = np.zeros(NBLK, np.int64)
    _OFS[1:] = np.cumsum(khat)[:-1]
    _OFSM = np.zeros(NBLK, np.int64)
    _OFSM[1:] = np.cumsum(khat + 1)[:-1]
    _KHAT = khat
    key = (sig, PHASES)
    if key not in _NC_CACHE:
        _NC_CACHE[key] = _build(sig, JTOT, MTOT)
    return _NC_CACHE[key], in_maps, tid


def kernel(**inputs):
    nc, in_maps, tid = _prepare(inputs)
    res = run_bass_kernel_spmd(nc, in_maps, list(range(NCORES)))
    full = np.concatenate([res.results[c]["out"] for c in range(NCORES)],
                          axis=0)
    return np.ascontiguousarray(full[tid[:N]]).astype(np.float32)
